# revision 5
# baseline (speedup 1.0000x reference)
"""BottomPool (cumulative max along H) Trainium2 Bass kernel.

Full input x: (16, 256, 128, 128) fp32. out[b,c,h,w] = max_{h'<=h} x[b,c,h',w].

Strategy: data-parallel over the 4096 (b,c) planes -> 512 planes per core.
Per core, planes are mapped [partition p in 0..127] x [q in 0..3] with
plane = q*128 + p. Device IO is bf16 (host casts fp32<->bf16), halving the
HBM traffic vs fp32: per-core 16.8MB read + 16.8MB write at the ~358 GB/s
per-core HBM cap -> ~94us roofline. bf16 keeps max rel err ~4e-3 uniformly
(fp16 subnormals near the harness' 1e-6 denom floor would not).

The cummax is a serial per-row chain, column-split across two engines so
the two independent chains run concurrently: DVE handles q in [0, qt-pool_q)
([128, 3*128] tensor_max per row, 2x bf16 mode), Pool/GpSimd handles the
rest ([128, 128] per row). Deep input buffering keeps the loads running
ahead of the chain so HBM streams continuously; small edge tiles start the
chain early and shrink the final store. Loads issue on nc.sync (SP HWDGE
ring); stores on nc.scalar (ACT ring). No transposes, no cross-core
communication.
"""

import numpy as np
import ml_dtypes

import concourse.tile as tile
from concourse import bacc, mybir
from concourse.bass_utils import run_bass_kernel_spmd

N_CORES = 8
B, C, H, W = 16, 256, 128, 128
P = 128  # SBUF partitions
PLANES_PER_CORE = (B * C) // N_CORES  # 512
BF16 = ml_dtypes.bfloat16


def build_module(planes=PLANES_PER_CORE, h=H, w=W, hs=8, qt=4,
                 n_cores=N_CORES, bufs_in=8, bufs_out=4,
                 store_engine="scalar", hsegs=None, pool_q=0):
    """Build + compile the per-core Bass module (same program on all cores).

    Layout: plane = q*128 + p; tiles are [128, qt, seg, w] bf16. The DMA
    descriptor contiguous chunk is seg*w*2 bytes. pool_q of the qt q-groups
    run their row chain on GpSimd/Pool, the rest on DVE — two independent
    serial recurrences that only join at each tile's store.
    """
    q = planes // P
    assert planes % P == 0 and q % qt == 0
    nq = q // qt
    if hsegs is None:
        # Small edge tiles start the chain early and shrink the final
        # store; big mid tiles keep DMA descriptor chunks at 3-6KB.
        hsegs = [4, 4, 8, 16, 24, 24, 24, 16, 4, 4]
    assert sum(hsegs) == h, (hsegs, h)
    assert 0 <= pool_q < qt
    nc = bacc.Bacc(
        "TRN2", target_bir_lowering=False, debug=False, num_devices=n_cores
    )
    x = nc.dram_tensor(
        "x", [planes, h, w], mybir.dt.bfloat16, kind="ExternalInput"
    ).ap()
    y = nc.dram_tensor(
        "y", [planes, h, w], mybir.dt.bfloat16, kind="ExternalOutput"
    ).ap()
    xv = x.rearrange("(q p) h w -> p q h w", p=P)
    yv = y.rearrange("(q p) h w -> p q h w", p=P)

    # (engine, q-slice) pairs owning independent column chains
    dve_q = qt - pool_q
    with tile.TileContext(nc) as tc:
        store_eng = getattr(nc, store_engine)
        chains = [("vector", 0, dve_q)]
        if pool_q:
            chains.append(("gpsimd", dve_q, qt))
        with (
            tc.tile_pool(name="pin", bufs=bufs_in) as pin,
            tc.tile_pool(name="pout", bufs=bufs_out) as pout,
        ):
            for qg in range(nq):
                qlo, qhi = qg * qt, (qg + 1) * qt
                prev = {name: None for name, _, _ in chains}
                h0 = 0
                for seg in hsegs:
                    tin = pin.tile([P, qt, seg, w], mybir.dt.bfloat16)
                    nc.sync.dma_start(
                        tin[:], xv[:, qlo:qhi, h0:h0 + seg, :]
                    )
                    tout = pout.tile([P, qt, seg, w], mybir.dt.bfloat16)
                    for hh in range(seg):
                        for name, elo, ehi in chains:
                            eng = getattr(nc, name)
                            cur = tin[:, elo:ehi, hh, :]
                            o = tout[:, elo:ehi, hh, :]
                            if prev[name] is None:
                                eng.tensor_copy(o, cur)
                            else:
                                eng.tensor_max(o, cur, prev[name])
                            prev[name] = o
                    store_eng.dma_start(
                        yv[:, qlo:qhi, h0:h0 + seg, :], tout[:]
                    )
                    h0 += seg
    nc.compile()
    return nc


_NC_CACHE = {}


def _get_module():
    if "nc" not in _NC_CACHE:
        _NC_CACHE["nc"] = build_module()
    return _NC_CACHE["nc"]


def kernel(x: np.ndarray) -> np.ndarray:
    assert x.shape == (B, C, H, W), x.shape
    x16 = np.ascontiguousarray(np.asarray(x, dtype=np.float32)).astype(BF16)
    flat = x16.reshape(B * C, H, W)
    in_maps = [
        {"x": flat[k * PLANES_PER_CORE:(k + 1) * PLANES_PER_CORE]}
        for k in range(N_CORES)
    ]
    nc = _get_module()
    res = run_bass_kernel_spmd(nc, in_maps, list(range(N_CORES)))
    out = np.concatenate([r["y"] for r in res.results], axis=0)
    return out.astype(np.float32).reshape(B, C, H, W)


# revision 6
# speedup vs baseline: 1.0758x; 1.0758x over previous
"""BottomPool (cumulative max along H) Trainium2 Bass kernel.

Full input x: (16, 256, 128, 128) fp32. out[b,c,h,w] = max_{h'<=h} x[b,c,h',w].

Strategy: data-parallel over the 4096 (b,c) planes -> 512 planes per core.
Per core, planes are mapped [partition p in 0..127] x [q in 0..3] with
plane = q*128 + p. Device IO is bf16 (host casts fp32<->bf16), halving the
HBM traffic vs fp32: per-core 16.8MB read + 16.8MB write at the ~358 GB/s
per-core HBM cap -> ~94us roofline. bf16 keeps max rel err ~4e-3 uniformly
(fp16 subnormals near the harness' 1e-6 denom floor would not).

The cummax is a serial per-row chain, column-split across two engines so
the two independent chains run concurrently: DVE handles q in [0, qt-pool_q)
([128, 3*128] tensor_max per row, 2x bf16 mode), Pool/GpSimd handles the
rest ([128, 128] per row). Deep input buffering keeps the loads running
ahead of the chain so HBM streams continuously; small edge tiles start the
chain early and shrink the final store. Loads issue on nc.sync (SP HWDGE
ring); stores on nc.scalar (ACT ring). No transposes, no cross-core
communication.
"""

import numpy as np
import ml_dtypes

import concourse.tile as tile
from concourse import bacc, mybir
from concourse.bass_utils import run_bass_kernel_spmd

N_CORES = 8
B, C, H, W = 16, 256, 128, 128
P = 128  # SBUF partitions
PLANES_PER_CORE = (B * C) // N_CORES  # 512
BF16 = ml_dtypes.bfloat16


def build_module(planes=PLANES_PER_CORE, h=H, w=W, hs=8, qt=4,
                 n_cores=N_CORES, bufs_in=8, bufs_out=4,
                 store_engine="scalar", hsegs=None, pool_q=0):
    """Build + compile the per-core Bass module (same program on all cores).

    Layout: plane = q*128 + p; tiles are [128, qt, seg, w] bf16. The DMA
    descriptor contiguous chunk is seg*w*2 bytes. pool_q of the qt q-groups
    run their row chain on GpSimd/Pool, the rest on DVE — two independent
    serial recurrences that only join at each tile's store.
    """
    q = planes // P
    assert planes % P == 0 and q % qt == 0
    nq = q // qt
    if hsegs is None:
        # Small head tiles start the DVE chain early; 32-row bulk tiles
        # keep DMA descriptor chunks at 8KB (per-descriptor fixed cost
        # dominates below ~4KB); tiny tail tiles shrink the final drain.
        hsegs = [8, 16, 32, 32, 32, 4, 4]
    assert sum(hsegs) == h, (hsegs, h)
    assert 0 <= pool_q < qt
    # Skip Bass.__init__'s four const-tile memsets: nothing in this kernel
    # reads const_aps, and they burn ~0.4us on GpSimd at startup.
    from concourse import bass as _bass
    _iface = _bass.BassSharedVectorInterface
    _orig_memset = _iface.memset

    def _memset_skip_consts(self, ap, value, *a, **k):
        name = getattr(getattr(ap, "tensor", None), "name", "")
        if isinstance(name, str) and name.startswith("const-"):
            return None
        return _orig_memset(self, ap, value, *a, **k)

    _iface.memset = _memset_skip_consts
    try:
        nc = bacc.Bacc(
            "TRN2", target_bir_lowering=False, debug=False, num_devices=n_cores
        )
    finally:
        _iface.memset = _orig_memset
    x = nc.dram_tensor(
        "x", [planes, h, w], mybir.dt.bfloat16, kind="ExternalInput"
    ).ap()
    y = nc.dram_tensor(
        "y", [planes, h, w], mybir.dt.bfloat16, kind="ExternalOutput"
    ).ap()
    xv = x.rearrange("(q p) h w -> p q h w", p=P)
    yv = y.rearrange("(q p) h w -> p q h w", p=P)

    # (engine, q-slice) pairs owning independent column chains
    dve_q = qt - pool_q
    with tile.TileContext(nc) as tc:
        store_eng = getattr(nc, store_engine)
        chains = [("vector", 0, dve_q)]
        if pool_q:
            chains.append(("gpsimd", dve_q, qt))
        with (
            tc.tile_pool(name="pin", bufs=bufs_in) as pin,
            tc.tile_pool(name="pout", bufs=bufs_out) as pout,
        ):
            for qg in range(nq):
                qlo, qhi = qg * qt, (qg + 1) * qt
                prev = {name: None for name, _, _ in chains}
                h0 = 0
                for seg in hsegs:
                    tin = pin.tile([P, qt, seg, w], mybir.dt.bfloat16)
                    nc.sync.dma_start(
                        tin[:], xv[:, qlo:qhi, h0:h0 + seg, :]
                    )
                    tout = pout.tile([P, qt, seg, w], mybir.dt.bfloat16)
                    for hh in range(seg):
                        for name, elo, ehi in chains:
                            eng = getattr(nc, name)
                            cur = tin[:, elo:ehi, hh, :]
                            o = tout[:, elo:ehi, hh, :]
                            if prev[name] is None:
                                eng.tensor_copy(o, cur)
                            else:
                                eng.tensor_max(o, cur, prev[name])
                            prev[name] = o
                    store_eng.dma_start(
                        yv[:, qlo:qhi, h0:h0 + seg, :], tout[:]
                    )
                    h0 += seg
    nc.compile()
    return nc


_NC_CACHE = {}


def _get_module():
    if "nc" not in _NC_CACHE:
        _NC_CACHE["nc"] = build_module()
    return _NC_CACHE["nc"]


def kernel(x: np.ndarray) -> np.ndarray:
    assert x.shape == (B, C, H, W), x.shape
    x16 = np.ascontiguousarray(np.asarray(x, dtype=np.float32)).astype(BF16)
    flat = x16.reshape(B * C, H, W)
    in_maps = [
        {"x": flat[k * PLANES_PER_CORE:(k + 1) * PLANES_PER_CORE]}
        for k in range(N_CORES)
    ]
    nc = _get_module()
    res = run_bass_kernel_spmd(nc, in_maps, list(range(N_CORES)))
    out = np.concatenate([r["y"] for r in res.results], axis=0)
    return out.astype(np.float32).reshape(B, C, H, W)


# revision 7
# speedup vs baseline: 1.1744x; 1.0917x over previous
"""BottomPool (cumulative max along H) Trainium2 Bass kernel.

Full input x: (16, 256, 128, 128) fp32. out[b,c,h,w] = max_{h'<=h} x[b,c,h',w].

Strategy: data-parallel over the 4096 (b,c) planes -> 512 planes per core.
Per core, planes are mapped [partition p in 0..127] x [q in 0..3] with
plane = q*128 + p. Device IO is bf16 (host casts fp32<->bf16), halving the
HBM traffic vs fp32: per-core 16.8MB read + 16.8MB write at the ~358 GB/s
per-core HBM cap -> ~94us roofline. bf16 keeps max rel err ~4e-3 uniformly
(fp16 subnormals near the harness' 1e-6 denom floor would not).

The cummax is a serial per-row chain, column-split across two engines so
the two independent chains run concurrently: DVE handles q in [0, qt-pool_q)
([128, 3*128] tensor_max per row, 2x bf16 mode), Pool/GpSimd handles the
rest ([128, 128] per row). Deep input buffering keeps the loads running
ahead of the chain so HBM streams continuously; small edge tiles start the
chain early and shrink the final store. Loads issue on nc.sync (SP HWDGE
ring); stores on nc.scalar (ACT ring). No transposes, no cross-core
communication.
"""

import numpy as np
import ml_dtypes

import concourse.tile as tile
from concourse import bacc, mybir
from concourse.bass_utils import run_bass_kernel_spmd

N_CORES = 8
B, C, H, W = 16, 256, 128, 128
P = 128  # SBUF partitions
PLANES_PER_CORE = (B * C) // N_CORES  # 512
BF16 = ml_dtypes.bfloat16


def build_module(planes=PLANES_PER_CORE, h=H, w=W, hs=8, qt=4,
                 n_cores=N_CORES, bufs_in=8, bufs_out=4,
                 store_engine="scalar", hsegs=None, pool_q=0):
    """Build + compile the per-core Bass module (same program on all cores).

    Layout: plane = q*128 + p; tiles are [128, qt, seg, w] bf16. The DMA
    descriptor contiguous chunk is seg*w*2 bytes. pool_q of the qt q-groups
    run their row chain on GpSimd/Pool, the rest on DVE — two independent
    serial recurrences that only join at each tile's store.
    """
    q = planes // P
    assert planes % P == 0 and q % qt == 0
    nq = q // qt
    if hsegs is None:
        # Small head tiles start the DVE chain early; 32-row bulk tiles
        # keep DMA descriptor chunks at 8KB (per-descriptor fixed cost
        # dominates below ~4KB); tiny tail tiles shrink the final drain.
        hsegs = [8, 16, 32, 32, 32, 4, 4]
    assert sum(hsegs) == h, (hsegs, h)
    assert 0 <= pool_q < qt
    # Skip Bass.__init__'s four const-tile memsets: nothing in this kernel
    # reads const_aps, and they burn ~0.4us on GpSimd at startup.
    from concourse import bass as _bass
    _patched = []
    for _cls in (_bass.BassEitherVectorEngine, _bass.BassGpSimd):
        if "memset" in _cls.__dict__:
            _patched.append((_cls, _cls.__dict__["memset"]))

    def _make_skip(orig):
        def _memset_skip_consts(self, ap, value, *a, **k):
            name = getattr(getattr(ap, "tensor", None), "name", "")
            if isinstance(name, str) and name.startswith("const-"):
                return None
            return orig(self, ap, value, *a, **k)
        return _memset_skip_consts

    for _cls, _orig in _patched:
        _cls.memset = _make_skip(_orig)
    try:
        nc = bacc.Bacc(
            "TRN2", target_bir_lowering=False, debug=False, num_devices=n_cores
        )
    finally:
        for _cls, _orig in _patched:
            _cls.memset = _orig
    x = nc.dram_tensor(
        "x", [planes, h, w], mybir.dt.bfloat16, kind="ExternalInput"
    ).ap()
    y = nc.dram_tensor(
        "y", [planes, h, w], mybir.dt.bfloat16, kind="ExternalOutput"
    ).ap()
    xv = x.rearrange("(q p) h w -> p q h w", p=P)
    yv = y.rearrange("(q p) h w -> p q h w", p=P)

    # (engine, q-slice) pairs owning independent column chains
    dve_q = qt - pool_q
    with tile.TileContext(nc) as tc:
        store_eng = getattr(nc, store_engine)
        chains = [("vector", 0, dve_q)]
        if pool_q:
            chains.append(("gpsimd", dve_q, qt))
        with (
            tc.tile_pool(name="pin", bufs=bufs_in) as pin,
            tc.tile_pool(name="pout", bufs=bufs_out) as pout,
        ):
            for qg in range(nq):
                qlo, qhi = qg * qt, (qg + 1) * qt
                prev = {name: None for name, _, _ in chains}
                h0 = 0
                for seg in hsegs:
                    tin = pin.tile([P, qt, seg, w], mybir.dt.bfloat16)
                    nc.sync.dma_start(
                        tin[:], xv[:, qlo:qhi, h0:h0 + seg, :]
                    )
                    tout = pout.tile([P, qt, seg, w], mybir.dt.bfloat16)
                    for hh in range(seg):
                        for name, elo, ehi in chains:
                            eng = getattr(nc, name)
                            cur = tin[:, elo:ehi, hh, :]
                            o = tout[:, elo:ehi, hh, :]
                            if prev[name] is None:
                                eng.tensor_copy(o, cur)
                            else:
                                eng.tensor_max(o, cur, prev[name])
                            prev[name] = o
                    store_eng.dma_start(
                        yv[:, qlo:qhi, h0:h0 + seg, :], tout[:]
                    )
                    h0 += seg
    nc.compile()
    return nc


_NC_CACHE = {}


def _get_module():
    if "nc" not in _NC_CACHE:
        _NC_CACHE["nc"] = build_module()
    return _NC_CACHE["nc"]


def kernel(x: np.ndarray) -> np.ndarray:
    assert x.shape == (B, C, H, W), x.shape
    x16 = np.ascontiguousarray(np.asarray(x, dtype=np.float32)).astype(BF16)
    flat = x16.reshape(B * C, H, W)
    in_maps = [
        {"x": flat[k * PLANES_PER_CORE:(k + 1) * PLANES_PER_CORE]}
        for k in range(N_CORES)
    ]
    nc = _get_module()
    res = run_bass_kernel_spmd(nc, in_maps, list(range(N_CORES)))
    out = np.concatenate([r["y"] for r in res.results], axis=0)
    return out.astype(np.float32).reshape(B, C, H, W)


# revision 8
# speedup vs baseline: 1.4612x; 1.2442x over previous
"""BottomPool (cumulative max along H) Trainium2 Bass kernel.

Full input x: (16, 256, 128, 128) fp32. out[b,c,h,w] = max_{h'<=h} x[b,c,h',w].

Strategy: data-parallel over the 4096 (b,c) planes -> 512 planes per core.
Per core, planes are mapped [partition p in 0..127] x [q in 0..3] with
plane = q*128 + p. Device IO is bf16 (host casts fp32<->bf16), halving the
HBM traffic vs fp32 (rel err ~4e-3, well under the 2e-2 gate; fp16 would
not survive the harness' 1e-6 denom floor near zero).

Phase-separated schedule: the whole 16.8MB per-core input is prefetched
into SBUF (it fits), with tile loads issued in REVERSE order so the chain's
first row op — which depends on tile 0, the last load to land — starts only
once everything is resident. The cummax then runs as an uninterrupted
serial chain of in-place [128, 4*128] DVE tensor_max ops (row 0 needs no
op at all: out == in), and each tile's store streams out behind the chain
on the ACT ring. Tile sizes taper at the tail so the final store drains in
~1.5us after the last row op. 32-row bulk tiles keep DMA descriptor chunks
at 8KB (per-descriptor fixed cost dominates below ~4KB).
"""

import numpy as np
import ml_dtypes

import concourse.tile as tile
from concourse import bacc, mybir
from concourse.bass_utils import run_bass_kernel_spmd

N_CORES = 8
B, C, H, W = 16, 256, 128, 128
P = 128  # SBUF partitions
PLANES_PER_CORE = (B * C) // N_CORES  # 512
BF16 = ml_dtypes.bfloat16


def _make_bacc(n_cores):
    """Bacc with Bass.__init__'s four const-tile memsets skipped: nothing
    in this kernel reads const_aps, and the memsets burn GpSimd time at
    startup before the first load can issue."""
    from concourse import bass as _bass
    patched = []
    for cls in (_bass.BassEitherVectorEngine, _bass.BassGpSimd):
        if "memset" in cls.__dict__:
            patched.append((cls, cls.__dict__["memset"]))

    def make_skip(orig):
        def memset_skip_consts(self, ap, value, *a, **k):
            name = getattr(getattr(ap, "tensor", None), "name", "")
            if isinstance(name, str) and name.startswith("const-"):
                return None
            return orig(self, ap, value, *a, **k)
        return memset_skip_consts

    for cls, orig in patched:
        cls.memset = make_skip(orig)
    try:
        return bacc.Bacc(
            "TRN2", target_bir_lowering=False, debug=False, num_devices=n_cores
        )
    finally:
        for cls, orig in patched:
            cls.memset = orig


def build_module(planes=PLANES_PER_CORE, h=H, w=W, qt=4, n_cores=N_CORES,
                 store_engine="scalar", hsegs=None, reverse_loads=True):
    q = planes // P
    assert planes % P == 0 and q == qt
    if hsegs is None:
        hsegs = [32, 32, 32, 16, 8, 4, 4]
    assert sum(hsegs) == h, (hsegs, h)
    nc = _make_bacc(n_cores)
    x = nc.dram_tensor(
        "x", [planes, h, w], mybir.dt.bfloat16, kind="ExternalInput"
    ).ap()
    y = nc.dram_tensor(
        "y", [planes, h, w], mybir.dt.bfloat16, kind="ExternalOutput"
    ).ap()
    xv = x.rearrange("(q p) h w -> p q h w", p=P)
    yv = y.rearrange("(q p) h w -> p q h w", p=P)

    with tile.TileContext(nc):
        store_eng = getattr(nc, store_engine)
        tiles = []
        h0 = 0
        for i, seg in enumerate(hsegs):
            t = nc.alloc_sbuf_tensor(
                f"tin{i}", [P, qt, seg, w], mybir.dt.bfloat16
            )
            tiles.append((t.ap(), h0, seg))
            h0 += seg
        # Prefetch everything; reversed issue order means tile 0 lands
        # last, so the first chain op implicitly waits for full residency.
        order = reversed(tiles) if reverse_loads else tiles
        for tv, h0, seg in order:
            nc.sync.dma_start(tv[:], xv[:, :, h0:h0 + seg, :])
        prev = None
        for tv, h0, seg in tiles:
            for hh in range(seg):
                cur = tv[:, :, hh, :]
                if prev is not None:
                    nc.vector.tensor_max(cur, cur, prev)
                prev = cur
            store_eng.dma_start(yv[:, :, h0:h0 + seg, :], tv[:])
    nc.compile()
    return nc


_NC_CACHE = {}


def _get_module():
    if "nc" not in _NC_CACHE:
        _NC_CACHE["nc"] = build_module()
    return _NC_CACHE["nc"]


def kernel(x: np.ndarray) -> np.ndarray:
    assert x.shape == (B, C, H, W), x.shape
    x16 = np.ascontiguousarray(np.asarray(x, dtype=np.float32)).astype(BF16)
    flat = x16.reshape(B * C, H, W)
    in_maps = [
        {"x": flat[k * PLANES_PER_CORE:(k + 1) * PLANES_PER_CORE]}
        for k in range(N_CORES)
    ]
    nc = _get_module()
    res = run_bass_kernel_spmd(nc, in_maps, list(range(N_CORES)))
    out = np.concatenate([r["y"] for r in res.results], axis=0)
    return out.astype(np.float32).reshape(B, C, H, W)


# revision 10
# speedup vs baseline: 1.5158x; 1.0374x over previous
"""BottomPool (cumulative max along H) Trainium2 Bass kernel.

Full input x: (16, 256, 128, 128) fp32. out[b,c,h,w] = max_{h'<=h} x[b,c,h',w].

Strategy: data-parallel over the 4096 (b,c) planes -> 512 planes per core.
Per core, planes are mapped [partition p in 0..127] x [q in 0..3] with
plane = q*128 + p. Device IO is bf16 (host casts fp32<->bf16), halving the
HBM traffic vs fp32 (rel err ~4e-3, well under the 2e-2 gate; fp16 would
not survive the harness' 1e-6 denom floor near zero).

Phase-separated schedule: the whole 16.8MB per-core input is prefetched
into SBUF (it fits), with tile loads issued in REVERSE order so the chain's
first row op — which depends on tile 0, the last load to land — starts only
once everything is resident. The cummax then runs as an uninterrupted
serial chain of in-place [128, 4*128] DVE tensor_max ops (row 0 needs no
op at all: out == in), and each tile's store streams out behind the chain
on the ACT ring. Tile sizes taper at the tail so the final store drains in
~1.5us after the last row op. 32-row bulk tiles keep DMA descriptor chunks
at 8KB (per-descriptor fixed cost dominates below ~4KB).
"""

import numpy as np
import ml_dtypes

import concourse.tile as tile
from concourse import bacc, mybir
from concourse.bass_utils import run_bass_kernel_spmd

N_CORES = 8
B, C, H, W = 16, 256, 128, 128
P = 128  # SBUF partitions
PLANES_PER_CORE = (B * C) // N_CORES  # 512
BF16 = ml_dtypes.bfloat16


def _make_bacc(n_cores):
    """Bacc with Bass.__init__'s four const-tile memsets skipped: nothing
    in this kernel reads const_aps, and the memsets burn GpSimd time at
    startup before the first load can issue."""
    from concourse import bass as _bass
    patched = []
    for cls in (_bass.BassEitherVectorEngine, _bass.BassGpSimd):
        if "memset" in cls.__dict__:
            patched.append((cls, cls.__dict__["memset"]))

    def make_skip(orig):
        def memset_skip_consts(self, ap, value, *a, **k):
            name = getattr(getattr(ap, "tensor", None), "name", "")
            if isinstance(name, str) and name.startswith("const-"):
                return None
            return orig(self, ap, value, *a, **k)
        return memset_skip_consts

    for cls, orig in patched:
        cls.memset = make_skip(orig)
    try:
        return bacc.Bacc(
            "TRN2", target_bir_lowering=False, debug=False, num_devices=n_cores
        )
    finally:
        for cls, orig in patched:
            cls.memset = orig


def build_module(planes=PLANES_PER_CORE, h=H, w=W, qt=4, n_cores=N_CORES,
                 store_engine="scalar", hsegs=None, reverse_loads=True,
                 store_segs=None):
    q = planes // P
    assert planes % P == 0 and q == qt
    if hsegs is None:
        hsegs = [32, 32, 32, 32]
    if store_segs is None:
        # Store chunks within each load tile: stores start streaming a few
        # us into the chain and the tapered tail drains in ~1.5us after
        # the last row op.
        store_segs = {len(hsegs) - 1: [16, 8, 4, 4]}
    assert sum(hsegs) == h, (hsegs, h)
    nc = _make_bacc(n_cores)
    x = nc.dram_tensor(
        "x", [planes, h, w], mybir.dt.bfloat16, kind="ExternalInput"
    ).ap()
    y = nc.dram_tensor(
        "y", [planes, h, w], mybir.dt.bfloat16, kind="ExternalOutput"
    ).ap()
    xv = x.rearrange("(q p) h w -> p q h w", p=P)
    yv = y.rearrange("(q p) h w -> p q h w", p=P)

    with tile.TileContext(nc):
        store_eng = getattr(nc, store_engine)
        tiles = []
        h0 = 0
        for i, seg in enumerate(hsegs):
            t = nc.alloc_sbuf_tensor(
                f"tin{i}", [P, qt, seg, w], mybir.dt.bfloat16
            )
            tiles.append((t.ap(), h0, seg))
            h0 += seg
        # Prefetch everything; reversed issue order means tile 0 lands
        # last, so the first chain op implicitly waits for full residency.
        order = reversed(tiles) if reverse_loads else tiles
        for tv, h0, seg in order:
            nc.sync.dma_start(tv[:], xv[:, :, h0:h0 + seg, :])
        prev = None
        for i, (tv, h0, seg) in enumerate(tiles):
            ssegs = store_segs.get(i, [min(16, seg)] * (seg // min(16, seg)))
            assert sum(ssegs) == seg, (i, ssegs, seg)
            s0 = 0
            for sseg in ssegs:
                for hh in range(s0, s0 + sseg):
                    cur = tv[:, :, hh, :]
                    if prev is not None:
                        nc.vector.tensor_max(cur, cur, prev)
                    prev = cur
                store_eng.dma_start(
                    yv[:, :, h0 + s0:h0 + s0 + sseg, :],
                    tv[:, :, s0:s0 + sseg, :],
                )
                s0 += sseg
    nc.compile()
    return nc


_NC_CACHE = {}


def _get_module():
    if "nc" not in _NC_CACHE:
        _NC_CACHE["nc"] = build_module()
    return _NC_CACHE["nc"]


def kernel(x: np.ndarray) -> np.ndarray:
    assert x.shape == (B, C, H, W), x.shape
    x16 = np.ascontiguousarray(np.asarray(x, dtype=np.float32)).astype(BF16)
    flat = x16.reshape(B * C, H, W)
    in_maps = [
        {"x": flat[k * PLANES_PER_CORE:(k + 1) * PLANES_PER_CORE]}
        for k in range(N_CORES)
    ]
    nc = _get_module()
    res = run_bass_kernel_spmd(nc, in_maps, list(range(N_CORES)))
    out = np.concatenate([r["y"] for r in res.results], axis=0)
    return out.astype(np.float32).reshape(B, C, H, W)


# revision 11
# speedup vs baseline: 1.5298x; 1.0092x over previous
"""BottomPool (cumulative max along H) Trainium2 Bass kernel.

Full input x: (16, 256, 128, 128) fp32. out[b,c,h,w] = max_{h'<=h} x[b,c,h',w].

Strategy: data-parallel over the 4096 (b,c) planes -> 512 planes per core.
Per core, planes are mapped [partition p in 0..127] x [q in 0..3] with
plane = q*128 + p. Device IO is bf16 (host casts fp32<->bf16), halving the
HBM traffic vs fp32 (rel err ~4e-3, well under the 2e-2 gate; fp16 would
not survive the harness' 1e-6 denom floor near zero).

Phase-separated schedule: the whole 16.8MB per-core input is prefetched
into SBUF (it fits), with tile loads issued in REVERSE order so the chain's
first row op — which depends on tile 0, the last load to land — starts only
once everything is resident. The cummax then runs as an uninterrupted
serial chain of in-place [128, 4*128] DVE tensor_max ops (row 0 needs no
op at all: out == in), and each tile's store streams out behind the chain
on the ACT ring. Tile sizes taper at the tail so the final store drains in
~1.5us after the last row op. 32-row bulk tiles keep DMA descriptor chunks
at 8KB (per-descriptor fixed cost dominates below ~4KB).
"""

import numpy as np
import ml_dtypes

import concourse.tile as tile
from concourse import bacc, mybir
from concourse.bass_utils import run_bass_kernel_spmd

N_CORES = 8
B, C, H, W = 16, 256, 128, 128
P = 128  # SBUF partitions
PLANES_PER_CORE = (B * C) // N_CORES  # 512
BF16 = ml_dtypes.bfloat16


def _make_bacc(n_cores):
    """Bacc with Bass.__init__'s four const-tile memsets skipped: nothing
    in this kernel reads const_aps, and the memsets burn GpSimd time at
    startup before the first load can issue."""
    from concourse import bass as _bass
    patched = []
    for cls in (_bass.BassEitherVectorEngine, _bass.BassGpSimd):
        if "memset" in cls.__dict__:
            patched.append((cls, cls.__dict__["memset"]))

    def make_skip(orig):
        def memset_skip_consts(self, ap, value, *a, **k):
            name = getattr(getattr(ap, "tensor", None), "name", "")
            if isinstance(name, str) and name.startswith("const-"):
                return None
            return orig(self, ap, value, *a, **k)
        return memset_skip_consts

    for cls, orig in patched:
        cls.memset = make_skip(orig)
    try:
        return bacc.Bacc(
            "TRN2", target_bir_lowering=False, debug=False, num_devices=n_cores
        )
    finally:
        for cls, orig in patched:
            cls.memset = orig


def build_module(planes=PLANES_PER_CORE, h=H, w=W, qt=4, n_cores=N_CORES,
                 store_engine="scalar", hsegs=None, reverse_loads=True,
                 store_segs=None):
    q = planes // P
    assert planes % P == 0 and q == qt
    if hsegs is None:
        hsegs = [32, 32, 32, 32]
    if store_segs is None:
        # Store chunks within each load tile: stores start streaming a few
        # us into the chain and the fine-grained tail drains right behind
        # the last row ops.
        store_segs = {len(hsegs) - 1: [8, 8, 8, 8]}
    store_segs = {int(k): v for k, v in store_segs.items()}
    assert sum(hsegs) == h, (hsegs, h)
    nc = _make_bacc(n_cores)
    x = nc.dram_tensor(
        "x", [planes, h, w], mybir.dt.bfloat16, kind="ExternalInput"
    ).ap()
    y = nc.dram_tensor(
        "y", [planes, h, w], mybir.dt.bfloat16, kind="ExternalOutput"
    ).ap()
    xv = x.rearrange("(q p) h w -> p q h w", p=P)
    yv = y.rearrange("(q p) h w -> p q h w", p=P)

    with tile.TileContext(nc):
        store_eng = getattr(nc, store_engine)
        tiles = []
        h0 = 0
        for i, seg in enumerate(hsegs):
            t = nc.alloc_sbuf_tensor(
                f"tin{i}", [P, qt, seg, w], mybir.dt.bfloat16
            )
            tiles.append((t.ap(), h0, seg))
            h0 += seg
        # Prefetch everything; reversed issue order means tile 0 lands
        # last, so the first chain op implicitly waits for full residency.
        order = reversed(tiles) if reverse_loads else tiles
        for tv, h0, seg in order:
            nc.sync.dma_start(tv[:], xv[:, :, h0:h0 + seg, :])
        prev = None
        for i, (tv, h0, seg) in enumerate(tiles):
            ssegs = store_segs.get(i, [min(16, seg)] * (seg // min(16, seg)))
            assert sum(ssegs) == seg, (i, ssegs, seg)
            s0 = 0
            for sseg in ssegs:
                for hh in range(s0, s0 + sseg):
                    cur = tv[:, :, hh, :]
                    if prev is not None:
                        nc.vector.tensor_max(cur, cur, prev)
                    prev = cur
                store_eng.dma_start(
                    yv[:, :, h0 + s0:h0 + s0 + sseg, :],
                    tv[:, :, s0:s0 + sseg, :],
                )
                s0 += sseg
    nc.compile()
    return nc


_NC_CACHE = {}


def _get_module():
    if "nc" not in _NC_CACHE:
        _NC_CACHE["nc"] = build_module()
    return _NC_CACHE["nc"]


def kernel(x: np.ndarray) -> np.ndarray:
    assert x.shape == (B, C, H, W), x.shape
    x16 = np.ascontiguousarray(np.asarray(x, dtype=np.float32)).astype(BF16)
    flat = x16.reshape(B * C, H, W)
    in_maps = [
        {"x": flat[k * PLANES_PER_CORE:(k + 1) * PLANES_PER_CORE]}
        for k in range(N_CORES)
    ]
    nc = _get_module()
    res = run_bass_kernel_spmd(nc, in_maps, list(range(N_CORES)))
    out = np.concatenate([r["y"] for r in res.results], axis=0)
    return out.astype(np.float32).reshape(B, C, H, W)
